# revision 60
# baseline (speedup 1.0000x reference)
"""GCNConv-style message passing kernel for Trainium2, 8 NeuronCores.

Computes (reference semantics):
    deg  = 1 + segment_sum(edge_weight, col)           # self-loop included
    dinv = deg ** -0.5
    h    = embs @ W
    out[t] = (sum_e norm_e * h[src_e] + dinv[t]^2 * h[t]) * X[t],
             norm_e = dinv[src_e] * ew_e * dinv[t]

Device formulation (matmul commutes past the segment sum):
    embs' = dinv[:, None] * embs                        (host, fp16)
    u[t]  = sum_{e: col=t} ew_e * embs'[src_e] + embs'[t]
    out[t] = (u[t] @ W) * (dinv[t] * X[t])

Sharding: targets split across 8 cores (12500 each). Edges are packed into
gather slots bucketed by (dest-block group of 8x128 targets, source bank of
25000 rows); within a (group, bank) segment the 8 dest blocks are packed
EXACTLY (no per-block padding) and segments are sized to the max count over
cores, rounded to 128. Edge source rows are fetched with dma_gather (int16
bank-local indices, wrap-16 x8-replicated for the 8 Q7 SWDGE cores). Per
128-slot chunk ONE 0/1 selection matrix S[slot, group_local_t] spanning the
chunk's dest-block window is built on DVE via TensorScalarPtr (is_equal of a
group-local iota against a per-partition fp32 scalar; 2-byte packed operands
get the 2x DVE mode) and PE-matmul-accumulated per dest block into PSUM
u^T[cin, t_loc]. PSUM accumulation groups are bank-granular (2KB zero
regions), so four dest blocks share one [128,512] bank whose group is opened
once by a full-bank zeroing matmul, accumulated with start=False, and closed
by the bank's last self-loop matmul. Self loops enter via an identity matmul
of a host-transposed embs' slice (rhs streamed [128, TPC] fp16); the final
(u @ W) * gX is computed transposed ([cout, t], gX streamed fp16
transposed), stored as fp16 [128, TPC], and un-transposed on the host.
"""

import numpy as np

import concourse.bacc as bacc
import concourse.tile as tile
from concourse import mybir
from concourse.bass_utils import run_bass_kernel_spmd

P = 128
SENTINEL = 2000.0


class _Cfg:
    def __init__(self, n, n_cores, bank_size, sb_group):
        self.N = n
        self.NCORES = n_cores
        self.TPC = n // n_cores              # targets per core
        assert self.TPC * n_cores == n
        self.NSB = -(-self.TPC // P)         # dest blocks of 128 per core
        self.BANK = bank_size                # gather bank rows (int16 < 32768)
        self.NBANK = -(-n // bank_size)
        assert bank_size <= 32768
        self.SBG = sb_group                  # dest blocks per group
        # tapered schedule: full groups, then 4/4/2/2/1-block groups so the
        # tail chains of the last big group finish before the final gathers
        taper = [4, 4, 2, 1, 1]
        full = (self.NSB - sum(taper)) // sb_group
        groups = [list(range(i * sb_group, (i + 1) * sb_group))
                  for i in range(full)]
        pos = full * sb_group
        rest = self.NSB - pos
        taper = taper[:]
        while sum(taper) < rest:
            taper.insert(0, min(sb_group, rest - sum(taper)))
        for t in taper:
            if pos >= self.NSB:
                break
            t = min(t, self.NSB - pos)
            groups.append(list(range(pos, pos + t)))
            pos += t
        self.GROUPS = groups
        self.GSTART = [g[0] for g in groups]
        self.NGRP = len(groups)


_REAL = _Cfg(n=100000, n_cores=8, bank_size=32768, sb_group=8)


def _host_prep(cfg, X, embs, W, edge_index, edge_weight):
    """Sort/bucket edges, build static segment/column schedule + per-core
    arrays."""
    N, TPC, NSB, BANK, NBANK, NCORES, SBG, NGRP = (
        cfg.N, cfg.TPC, cfg.NSB, cfg.BANK, cfg.NBANK, cfg.NCORES,
        cfg.SBG, cfg.NGRP)

    src = np.asarray(edge_index[0], dtype=np.int64)
    col = np.asarray(edge_index[1], dtype=np.int64)
    ew = np.asarray(edge_weight, dtype=np.float64)
    E = src.shape[0]

    deg = 1.0 + np.bincount(col, weights=ew, minlength=N)
    dinv = (1.0 / np.sqrt(deg)).astype(np.float32)

    embs16 = (dinv[:, None] * np.asarray(embs, np.float32)).astype(np.float16)
    gX = (dinv[:, None] * np.asarray(X, np.float32)).astype(np.float16)

    ew_ones = bool(np.all(np.asarray(edge_weight) == 1.0))
    ew32 = None if ew_ones else np.asarray(edge_weight, np.float32)

    core = col // TPC
    tl = ((col % TPC) % P).astype(np.float32)
    sb = (col % TPC) // P
    group_of = np.zeros(NSB, np.int64)
    gstart = np.zeros(NSB, np.int64)
    for g_, blks in enumerate(cfg.GROUPS):
        for s_ in blks:
            group_of[s_] = g_
            gstart[s_] = blks[0]
    gi = group_of[sb]
    bank = src // BANK
    srcl = (src - bank * BANK).astype(np.int16)

    # pairing: same-source edges within one (core, group, bank) segment
    # share a gather slot; the pair's second edge is applied via a second
    # one-hot (S2) against the same gathered row
    tlg_all = (col % TPC - gstart[sb] * P).astype(np.float32)
    NSEG = NCORES * NGRP * NBANK
    key_seg = (core * NGRP + gi) * NBANK + bank
    po_ = np.argsort(key_seg * np.int64(N) + src, kind="stable")
    seg_s = key_seg[po_]
    src_s = src[po_]
    sb_s = sb[po_]
    tlg_s = tlg_all[po_]
    srcl_s = srcl[po_]
    pk_s = seg_s * np.int64(N) + src_s
    new_grp = np.ones(E, bool)
    new_grp[1:] = pk_s[1:] != pk_s[:-1]
    gsi = np.cumsum(new_grp) - 1
    gstarts = np.nonzero(new_grp)[0]
    occ = np.arange(E) - gstarts[gsi]
    cnt_of = np.bincount(gsi)[gsi]
    is_pair = occ < 2 * (cnt_of // 2)

    pa = np.nonzero(is_pair & (occ % 2 == 0))[0]
    pb = pa + 1
    sing = np.nonzero(~is_pair)[0]
    # pairs ordered by (segment, sb of first edge)
    p_ord = np.lexsort((sb_s[pa], seg_s[pa]))
    pa = pa[p_ord]; pb = pb[p_ord]
    ps = seg_s[pa]
    pcnt = np.bincount(ps, minlength=NSEG)
    p0 = np.zeros(NSEG, np.int64); np.cumsum(pcnt[:-1], out=p0[1:])
    pair_rank = np.arange(len(pa)) - p0[ps]
    # singles ordered by (segment, sb)
    s_ord = np.lexsort((sb_s[sing], seg_s[sing]))
    sing = sing[s_ord]
    ss = seg_s[sing]
    scnt = np.bincount(ss, minlength=NSEG)
    s0 = np.zeros(NSEG, np.int64); np.cumsum(scnt[:-1], out=s0[1:])
    sing_rank = np.arange(len(sing)) - s0[ss]

    # paired region padded to a static 128-aligned size so the singles
    # start at the same chunk-aligned offset on every core (measured better
    # than unaligned: mixed boundary chunks widen S1 spans by more than the
    # padding costs)
    pc3_ = pcnt.reshape(NCORES, NGRP, NBANK)
    sc3_ = scnt.reshape(NCORES, NGRP, NBANK)
    pseg = -(-pc3_.max(axis=0) // P) * P              # [NGRP, NBANK]
    seg_exact = pseg + sc3_.max(axis=0)
    seg = -(-seg_exact // P) * P
    seg_off = np.zeros((NGRP, NBANK), np.int64)
    pos = 0
    for g in range(NGRP):
        for b in range(NBANK):
            seg_off[g, b] = pos
            pos += int(seg[g, b])
    slots_tot = pos

    seg_off_flat = np.zeros(NSEG, np.int64)
    cseg = np.arange(NSEG)
    gg = (cseg // NBANK) % NGRP
    bb = cseg % NBANK
    seg_off_flat[:] = seg_off[gg, bb]
    c_of_seg = cseg // (NGRP * NBANK)

    # absolute per-core slot index of pairs and singles
    pseg_flat = pseg[gg, bb]
    slotA = seg_off_flat[ps] + pair_rank
    slotS = seg_off_flat[ss] + pseg_flat[ss] + sing_rank
    coreA = c_of_seg[ps]
    coreS = c_of_seg[ss]

    IDX = np.zeros((NCORES, slots_tot), np.int16)
    IDX[coreA, slotA] = srcl_s[pa]
    IDX[coreS, slotS] = srcl_s[sing]

    # chunk schedule: A-side spans from (paired-by-sbA | singles-by-sb)
    # interval marking; B-side spans per chunk from the pair second edges
    maxnch = int(seg.max()) // P
    chunkmap = np.full((NGRP, NBANK, maxnch), -1, np.int64)
    chunks = [[] for _ in range(NGRP)]
    nchunks = 0
    sc4 = np.zeros((NCORES, NGRP, NBANK, NSB), np.int64)
    np.add.at(sc4, (coreS, gg[ss], bb[ss], sb_s[sing]), 1)
    pc4 = np.zeros((NCORES, NGRP, NBANK, NSB), np.int64)
    np.add.at(pc4, (c_of_seg[ps], gg[ps], bb[ps], sb_s[pa]), 1)
    pc3 = pcnt.reshape(NCORES, NGRP, NBANK)
    # per-(g,b,chunk) B-sb min/max over all cores
    jA = pair_rank // P
    keyBJ = (gg[ps] * NBANK + bb[ps]) * maxnch + jA
    bmin = np.full(NGRP * NBANK * maxnch, NSB + 1, np.int64)
    bmax = np.full(NGRP * NBANK * maxnch, -1, np.int64)
    np.minimum.at(bmin, keyBJ, sb_s[pb])
    np.maximum.at(bmax, keyBJ, sb_s[pb])
    for g in range(NGRP):
        sbs = cfg.GROUPS[g]
        for b in range(NBANK):
            nch = int(seg[g, b]) // P
            pres = np.zeros((nch, len(sbs)), bool)
            for c in range(NCORES):
                lo = 0
                for si, s in enumerate(sbs):
                    hi = lo + int(pc4[c, g, b, s])
                    if hi > lo:
                        pres[lo // P:(hi - 1) // P + 1, si] = True
                    lo = hi
                lo = int(pseg[g, b])
                for si, s in enumerate(sbs):
                    hi = lo + int(sc4[c, g, b, s])
                    if hi > lo:
                        pres[lo // P:(hi - 1) // P + 1, si] = True
                    lo = hi
            for j in range(nch):
                (idxs,) = np.nonzero(pres[j])
                if len(idxs) == 0:
                    continue
                sb_lo = sbs[int(idxs[0])]
                nspan = int(idxs[-1]) - int(idxs[0]) + 1
                k2 = (g * NBANK + b) * maxnch + j
                if bmax[k2] >= 0:
                    sb_lo2 = int(bmin[k2])
                    nspan2 = int(bmax[k2]) - sb_lo2 + 1
                else:
                    sb_lo2, nspan2 = 0, 0
                chunkmap[g, b, j] = nchunks
                chunks[g].append((b, j, sb_lo, nspan, nchunks,
                                  sb_lo2, nspan2))
                nchunks += 1

    TLOC = np.full((NCORES, nchunks, P), SENTINEL, np.float32)
    TLOC2 = np.full((NCORES, nchunks, P), SENTINEL, np.float32)

    def _fill(cores_, slots_, vals_, arr):
        segsl = slots_  # absolute slots; chunk via per-seg offsets
        g_ = np.zeros(len(slots_), np.int64)
        b_ = np.zeros(len(slots_), np.int64)
        # recover (g,b) from slot via searchsorted on seg_off boundaries
        bounds = np.array([int(seg_off[gx, bx]) for gx in range(NGRP)
                           for bx in range(NBANK)] + [slots_tot])
        si_ = np.searchsorted(bounds, slots_, side="right") - 1
        g_ = si_ // NBANK
        b_ = si_ % NBANK
        j_ = (slots_ - bounds[si_]) // P
        pos_ = (slots_ - bounds[si_]) % P
        ci_ = chunkmap[g_, b_, j_]
        assert (ci_ >= 0).all()
        arr[cores_, ci_, pos_] = vals_

    _fill(coreA, slotA, tlg_s[pa], TLOC)
    _fill(coreA, slotA, tlg_s[pb], TLOC2)
    _fill(coreS, slotS, tlg_s[sing], TLOC)
    EWA = None
    if not ew_ones:
        raise NotImplementedError("pairing path assumes unit edge weights")

    # pack gather indices: wrap-16, replicate to 128 partitions
    assert slots_tot % 16 == 0
    idx_packed = IDX.reshape(NCORES, slots_tot // 16, 16).transpose(0, 2, 1)
    idx_all = np.tile(idx_packed, (1, 8, 1)).astype(np.int16)

    tloc_all = TLOC.transpose(0, 2, 1).copy()         # [NCORES, P, ncols]
    tloc2_all = TLOC2.transpose(0, 2, 1).copy()
    ew_all = None if EWA is None else EWA.transpose(0, 2, 1).copy()

    iota16 = np.tile(np.arange(SBG * P, dtype=np.float16), (P, 1))
    ident16 = np.eye(P, dtype=np.float16)

    sched = dict(chunks=chunks, seg=seg, seg_exact=seg_exact,
                 seg_off=seg_off, slots_tot=slots_tot,
                 nchunks=nchunks, ew_ones=ew_ones)
    in_maps = []
    for c in range(NCORES):
        m = dict(
            embs16=embs16,
            w32=np.asarray(W, np.float32),
            gxT=np.ascontiguousarray(gX[c * TPC:(c + 1) * TPC].T),
            selfT=np.ascontiguousarray(embs16[c * TPC:(c + 1) * TPC].T),
            idxall=np.ascontiguousarray(idx_all[c]),
            tlocall=np.ascontiguousarray(tloc_all[c]),
            tloc2all=np.ascontiguousarray(tloc2_all[c]),
            iota16=iota16,
            ident16=ident16,
        )
        if ew_all is not None:
            m["ewall"] = np.ascontiguousarray(ew_all[c])
        in_maps.append(m)
    return sched, in_maps


def _build_program(cfg, sched):
    N, TPC, NSB, BANK, NBANK, SBG, NGRP = (
        cfg.N, cfg.TPC, cfg.NSB, cfg.BANK, cfg.NBANK, cfg.SBG, cfg.NGRP)
    chunks, seg, seg_exact, seg_off, slots_tot, nchunks, ew_ones = (
        sched["chunks"], sched["seg"], sched["seg_exact"], sched["seg_off"],
        sched["slots_tot"], sched["nchunks"], sched["ew_ones"])

    nc = bacc.Bacc("TRN2", target_bir_lowering=False, debug=False,
                   num_devices=cfg.NCORES)
    t_embs16 = nc.dram_tensor("embs16", [N, P], mybir.dt.float16,
                              kind="ExternalInput").ap()
    t_w = nc.dram_tensor("w32", [P, P], mybir.dt.float32,
                         kind="ExternalInput").ap()
    t_gxT = nc.dram_tensor("gxT", [P, TPC], mybir.dt.float16,
                           kind="ExternalInput").ap()
    t_selfT = nc.dram_tensor("selfT", [P, TPC], mybir.dt.float16,
                             kind="ExternalInput").ap()
    t_idx = nc.dram_tensor("idxall", [P, slots_tot // 16], mybir.dt.int16,
                           kind="ExternalInput").ap()
    t_tloc = nc.dram_tensor("tlocall", [P, nchunks], mybir.dt.float32,
                            kind="ExternalInput").ap()
    t_tloc2 = nc.dram_tensor("tloc2all", [P, nchunks], mybir.dt.float32,
                             kind="ExternalInput").ap()
    t_iota = nc.dram_tensor("iota16", [P, SBG * P], mybir.dt.float16,
                            kind="ExternalInput").ap()
    t_ident = nc.dram_tensor("ident16", [P, P], mybir.dt.float16,
                             kind="ExternalInput").ap()
    t_ew = None
    if not ew_ones:
        t_ew = nc.dram_tensor("ewall", [P, nchunks], mybir.dt.float32,
                              kind="ExternalInput").ap()
    t_outT = nc.dram_tensor("outT", [P, TPC], mybir.dt.float16,
                            kind="ExternalOutput").ap()

    with tile.TileContext(nc) as tc:
        with tc.tile_pool(name="const", bufs=1) as cpool, \
             tc.tile_pool(name="meta", bufs=1) as mpool, \
             tc.tile_pool(name="gpool", bufs=8) as gpool, \
             tc.tile_pool(name="spool", bufs=12) as spool, \
             tc.tile_pool(name="grp", bufs=3) as grp, \
             tc.tile_pool(name="xfer", bufs=6) as xfer, \
             tc.tile_pool(name="psu", bufs=4, space="PSUM") as psu, \
             tc.tile_pool(name="psb", bufs=4, space="PSUM") as psb:

            iota_t = cpool.tile([P, SBG * P], mybir.dt.float16)
            nc.sync.dma_start(out=iota_t, in_=t_iota)
            zeros_t = cpool.tile([P, 4 * P], mybir.dt.float16)
            nc.vector.memset(zeros_t[:, :], 0.0)
            ident_t = cpool.tile([P, P], mybir.dt.float16)
            nc.gpsimd.dma_start(out=ident_t, in_=t_ident)
            w_t = cpool.tile([P, P], mybir.dt.float32)
            nc.gpsimd.dma_start(out=w_t, in_=t_w)
            # idx loaded in pieces (separate tiles): a tiny head covering
            # only the first (group, bank) segment lets the first gather's
            # SWDGE generation start ~3us earlier; the rest follows in
            # group-aligned slabs behind tloc
            idx_cuts = [0, int(seg[0, 0]) // 16,
                        int(seg_off[2, 0]) // 16,
                        int(seg_off[6, 0]) // 16,
                        slots_tot // 16]
            idx_ts = []
            q0 = mpool.tile([P, idx_cuts[1]], mybir.dt.int16, name="idx_q0")
            nc.gpsimd.dma_start(out=q0, in_=t_idx[:, 0:idx_cuts[1]])
            idx_ts.append((0, q0))
            tloc_t = mpool.tile([P, nchunks], mybir.dt.float32)
            nc.gpsimd.dma_start(out=tloc_t, in_=t_tloc)
            tloc2_t = mpool.tile([P, nchunks], mybir.dt.float32)
            nc.gpsimd.dma_start(out=tloc2_t, in_=t_tloc2)
            for qi in range(1, len(idx_cuts) - 1):
                lo, hi = idx_cuts[qi], idx_cuts[qi + 1]
                q_t = mpool.tile([P, hi - lo], mybir.dt.int16,
                                 name=f"idx_q{qi}")
                nc.sync.dma_start(out=q_t, in_=t_idx[:, lo:hi])
                idx_ts.append((lo, q_t))

            def _idx_slice(o16, n16):
                for qi in range(len(idx_ts) - 1, -1, -1):
                    lo, q_t = idx_ts[qi]
                    if o16 >= lo:
                        return q_t[:, o16 - lo:o16 - lo + n16]
                raise AssertionError
            ew_t = None
            if t_ew is not None:
                ew_t = mpool.tile([P, nchunks], mybir.dt.float32)
                nc.sync.dma_start(out=ew_t, in_=t_ew)

            nch_max = int(seg.max()) // P
            for wi in range(8):
                warm_t = gpool.tile([P, nch_max, P], mybir.dt.float16,
                                    tag="g", name=f"warm_{wi}")
                nc.vector.memset(warm_t[:, :, :], 0.0)


            for g in range(NGRP):
                sbs = cfg.GROUPS[g]
                g0 = cfg.GSTART[g]
                t0g = g0 * P
                wg = min(len(sbs) * P, TPC - t0g)

                g_tiles = []
                for b in range(NBANK):
                    nch = int(seg[g, b]) // P
                    g_t = gpool.tile([P, nch, P], mybir.dt.float16, tag="g")
                    o16 = int(seg_off[g, b]) // 16
                    exact = int(seg_exact[g, b])
                    assert 0 < exact <= nch * P
                    # the very last segment is fetched in three pieces so the
                    # tail chunks' matmuls overlap the remaining transfers
                    npc = 3 if (g == NGRP - 1 and b == NBANK - 1) else 1
                    ccuts = sorted({0} | {min((-(-nch // npc)) * (i + 1), nch)
                                          for i in range(npc)})
                    for c0, c1 in zip(ccuts[:-1], ccuts[1:]):
                        ni = min(exact, c1 * P) - c0 * P
                        if ni <= 0:
                            continue
                        rows = min(BANK, N - b * BANK)
                        nc.gpsimd.dma_gather(
                            out_ap=g_t[:, c0:c1, :],
                            in_ap=t_embs16[b * BANK: b * BANK + rows, :],
                            idxs_ap=_idx_slice(o16 + c0 * 8, -(-ni // 16)),
                            num_idxs=ni,
                            num_idxs_reg=ni,
                            elem_size=P,
                            single_packet=False,
                        )
                    g_tiles.append(g_t)

                # stream loads AFTER the gather issues: they are only
                # consumed by this group's tail, and keeping them behind the
                # gathers in the DMA queue lets the final gathers finish
                # earlier
                selfT_t = grp.tile([P, wg], mybir.dt.float16, tag="self")
                nc.sync.dma_start(out=selfT_t, in_=t_selfT[:, t0g:t0g + wg])
                gxT_t = grp.tile([P, wg], mybir.dt.float16, tag="gx")
                nc.sync.dma_start(out=gxT_t, in_=t_gxT[:, t0g:t0g + wg])
                outT_t = grp.tile([P, wg], mybir.dt.float16, tag="out")

                nhalf = -(-len(sbs) // 4)
                pu_t = []
                for h in range(nhalf):
                    p_t = psu.tile([P, 4 * P], mybir.dt.float32, space="PSUM",
                                   tag="pu", name=f"psu_{g}_{h}")
                    # open the bank's single accumulation group, zeroing all
                    # four 128-col windows (PSUM groups are bank-granular)
                    nc.tensor.matmul(out=p_t[:, :], lhsT=ident_t,
                                     rhs=zeros_t, start=True, stop=False)
                    pu_t.append(p_t)

                def _uwin(s, pu_t=pu_t, g0=g0):
                    sbl = s - g0
                    return pu_t[sbl // 4], (sbl % 4) * P

                # last sb of each bank closes the group (stop=True)
                last_sb_of_half = {h: sbs[min(4 * h + 4, len(sbs)) - 1]
                                   for h in range(nhalf)}



                for (b, j, sb_lo, nspan, ci, sb_lo2, nspan2) in chunks[g]:
                    w0 = (sb_lo - g0) * P
                    ws = nspan * P
                    s_t = spool.tile([P, ws], mybir.dt.float16, tag="s",
                                     name=f"s_{g}_{b}_{j}")
                    nc.vector.tensor_scalar(
                        out=s_t, in0=iota_t[:, w0:w0 + ws],
                        scalar1=tloc_t[:, ci:ci + 1], scalar2=None,
                        op0=mybir.AluOpType.is_equal)
                    for k in range(nspan):
                        s = sb_lo + k
                        tw = min(P, TPC - s * P)
                        put, uoff = _uwin(s)
                        nc.tensor.matmul(
                            out=put[:, uoff:uoff + tw],
                            lhsT=g_tiles[b][:, j, :],
                            rhs=s_t[:, k * P:k * P + tw],
                            start=False, stop=False)
                    if nspan2 > 0:
                        w02 = (sb_lo2 - g0) * P
                        ws2 = nspan2 * P
                        s2_t = spool.tile([P, ws2], mybir.dt.float16,
                                          tag="s", name=f"s2_{g}_{b}_{j}")
                        nc.vector.tensor_scalar(
                            out=s2_t, in0=iota_t[:, w02:w02 + ws2],
                            scalar1=tloc2_t[:, ci:ci + 1], scalar2=None,
                            op0=mybir.AluOpType.is_equal)
                        for k in range(nspan2):
                            s = sb_lo2 + k
                            tw = min(P, TPC - s * P)
                            put, uoff = _uwin(s)
                            nc.tensor.matmul(
                                out=put[:, uoff:uoff + tw],
                                lhsT=g_tiles[b][:, j, :],
                                rhs=s2_t[:, k * P:k * P + tw],
                                start=False, stop=False)

                def _tail(g, sbs, _uwin, last_sb_of_half, selfT_t,
                          gxT_t, outT_t, t0g, wg, g0):
                    for s in sbs:
                        tw = min(P, TPC - s * P)
                        sbl = s - g0
                        # self loops: += embs'^T[:, t] via identity matmul
                        put, uoff = _uwin(s)
                        h = sbl // 4
                        nc.tensor.matmul(
                            out=put[:, uoff:uoff + tw],
                            lhsT=ident_t,
                            rhs=selfT_t[:, sbl * P: sbl * P + tw],
                            start=False, stop=(s == last_sb_of_half[h]))

                        u_t = xfer.tile([P, P], mybir.dt.float32, tag="u")
                        if g == NGRP - 1 and sbl % 2:
                            nc.vector.tensor_copy(out=u_t[:, :tw],
                                                  in_=put[:, uoff:uoff + tw])
                        else:
                            nc.scalar.copy(out=u_t[:, :tw],
                                           in_=put[:, uoff:uoff + tw])

                        pot = psb.tile([P, P], mybir.dt.float32, space="PSUM",
                                       tag="po", name=f"pso_{g}_{s}")
                        nc.tensor.matmul(out=pot[:, :tw], lhsT=w_t,
                                         rhs=u_t[:, :tw], start=True,
                                         stop=True)

                        nc.vector.tensor_tensor(
                            out=outT_t[:, sbl * P: sbl * P + tw],
                            in0=pot[:, :tw],
                            in1=gxT_t[:, sbl * P: sbl * P + tw],
                            op=mybir.AluOpType.mult)

                    nc.sync.dma_start(out=t_outT[:, t0g:t0g + wg],
                                      in_=outT_t)

                _tail(g, sbs, _uwin, last_sb_of_half, selfT_t, gxT_t,
                      outT_t, t0g, wg, g0)
    nc.compile()
    return nc


def kernel(X, embs, W, edge_index, edge_weight):
    cfg = _REAL
    sched, in_maps = _host_prep(cfg, X, embs, W, edge_index, edge_weight)
    nc = _build_program(cfg, sched)
    res = run_bass_kernel_spmd(nc, in_maps, list(range(cfg.NCORES)))
    out = np.concatenate(
        [res.results[c]["outT"].T for c in range(cfg.NCORES)], axis=0)
    return out.astype(np.float32)


# revision 61
# speedup vs baseline: 1.0123x; 1.0123x over previous
"""GCNConv-style message passing kernel for Trainium2, 8 NeuronCores.

Computes (reference semantics):
    deg  = 1 + segment_sum(edge_weight, col)           # self-loop included
    dinv = deg ** -0.5
    h    = embs @ W
    out[t] = (sum_e norm_e * h[src_e] + dinv[t]^2 * h[t]) * X[t],
             norm_e = dinv[src_e] * ew_e * dinv[t]

Device formulation (matmul commutes past the segment sum):
    embs' = dinv[:, None] * embs                        (host, fp16)
    u[t]  = sum_{e: col=t} ew_e * embs'[src_e] + embs'[t]
    out[t] = (u[t] @ W) * (dinv[t] * X[t])

Sharding: targets split across 8 cores (12500 each). Edges are packed into
gather slots bucketed by (dest-block group of 8x128 targets, source bank of
25000 rows); within a (group, bank) segment the 8 dest blocks are packed
EXACTLY (no per-block padding) and segments are sized to the max count over
cores, rounded to 128. Edge source rows are fetched with dma_gather (int16
bank-local indices, wrap-16 x8-replicated for the 8 Q7 SWDGE cores). Per
128-slot chunk ONE 0/1 selection matrix S[slot, group_local_t] spanning the
chunk's dest-block window is built on DVE via TensorScalarPtr (is_equal of a
group-local iota against a per-partition fp32 scalar; 2-byte packed operands
get the 2x DVE mode) and PE-matmul-accumulated per dest block into PSUM
u^T[cin, t_loc]. PSUM accumulation groups are bank-granular (2KB zero
regions), so four dest blocks share one [128,512] bank whose group is opened
once by a full-bank zeroing matmul, accumulated with start=False, and closed
by the bank's last self-loop matmul. Self loops enter via an identity matmul
of a host-transposed embs' slice (rhs streamed [128, TPC] fp16); the final
(u @ W) * gX is computed transposed ([cout, t], gX streamed fp16
transposed), stored as fp16 [128, TPC], and un-transposed on the host.
"""

import numpy as np

import concourse.bacc as bacc
import concourse.tile as tile
from concourse import mybir
from concourse.bass_utils import run_bass_kernel_spmd

P = 128
SENTINEL = 2000.0


class _Cfg:
    def __init__(self, n, n_cores, bank_size, sb_group):
        self.N = n
        self.NCORES = n_cores
        self.TPC = n // n_cores              # targets per core
        assert self.TPC * n_cores == n
        self.NSB = -(-self.TPC // P)         # dest blocks of 128 per core
        self.BANK = bank_size                # gather bank rows (int16 < 32768)
        self.NBANK = -(-n // bank_size)
        assert bank_size <= 32768
        self.SBG = sb_group                  # dest blocks per group
        # tapered schedule: full groups, then 4/4/2/2/1-block groups so the
        # tail chains of the last big group finish before the final gathers
        taper = [4, 4, 2, 1, 1]
        full = (self.NSB - sum(taper)) // sb_group
        groups = [list(range(i * sb_group, (i + 1) * sb_group))
                  for i in range(full)]
        pos = full * sb_group
        rest = self.NSB - pos
        taper = taper[:]
        while sum(taper) < rest:
            taper.insert(0, min(sb_group, rest - sum(taper)))
        for t in taper:
            if pos >= self.NSB:
                break
            t = min(t, self.NSB - pos)
            groups.append(list(range(pos, pos + t)))
            pos += t
        self.GROUPS = groups
        self.GSTART = [g[0] for g in groups]
        self.NGRP = len(groups)


_REAL = _Cfg(n=100000, n_cores=8, bank_size=32768, sb_group=8)


def _host_prep(cfg, X, embs, W, edge_index, edge_weight):
    """Sort/bucket edges, build static segment/column schedule + per-core
    arrays."""
    N, TPC, NSB, BANK, NBANK, NCORES, SBG, NGRP = (
        cfg.N, cfg.TPC, cfg.NSB, cfg.BANK, cfg.NBANK, cfg.NCORES,
        cfg.SBG, cfg.NGRP)

    src = np.asarray(edge_index[0], dtype=np.int64)
    col = np.asarray(edge_index[1], dtype=np.int64)
    ew = np.asarray(edge_weight, dtype=np.float64)
    E = src.shape[0]

    deg = 1.0 + np.bincount(col, weights=ew, minlength=N)
    dinv = (1.0 / np.sqrt(deg)).astype(np.float32)

    embs16 = (dinv[:, None] * np.asarray(embs, np.float32)).astype(np.float16)
    gX = (dinv[:, None] * np.asarray(X, np.float32)).astype(np.float16)

    ew_ones = bool(np.all(np.asarray(edge_weight) == 1.0))
    ew32 = None if ew_ones else np.asarray(edge_weight, np.float32)

    core = col // TPC
    tl = ((col % TPC) % P).astype(np.float32)
    sb = (col % TPC) // P
    group_of = np.zeros(NSB, np.int64)
    gstart = np.zeros(NSB, np.int64)
    for g_, blks in enumerate(cfg.GROUPS):
        for s_ in blks:
            group_of[s_] = g_
            gstart[s_] = blks[0]
    gi = group_of[sb]
    bank = src // BANK
    srcl = (src - bank * BANK).astype(np.int16)

    # pairing: same-source edges within one (core, group, bank) segment
    # share a gather slot; the pair's second edge is applied via a second
    # one-hot (S2) against the same gathered row
    tlg_all = (col % TPC - gstart[sb] * P).astype(np.float32)
    NSEG = NCORES * NGRP * NBANK
    key_seg = (core * NGRP + gi) * NBANK + bank
    po_ = np.argsort(key_seg * np.int64(N) + src, kind="stable")
    seg_s = key_seg[po_]
    src_s = src[po_]
    sb_s = sb[po_]
    tlg_s = tlg_all[po_]
    srcl_s = srcl[po_]
    pk_s = seg_s * np.int64(N) + src_s
    new_grp = np.ones(E, bool)
    new_grp[1:] = pk_s[1:] != pk_s[:-1]
    gsi = np.cumsum(new_grp) - 1
    gstarts = np.nonzero(new_grp)[0]
    occ = np.arange(E) - gstarts[gsi]
    cnt_of = np.bincount(gsi)[gsi]
    is_pair = occ < 2 * (cnt_of // 2)
    # pairing off in the small taper groups: their ceil128 paired-region
    # padding costs more descriptors than the few pairs save, and it puts
    # S2 work into the end-of-kernel drain
    big_grp = np.array([len(b_) >= SBG for b_ in cfg.GROUPS])
    g_of_seg_s = (seg_s // NBANK) % NGRP
    is_pair &= big_grp[g_of_seg_s]

    pa = np.nonzero(is_pair & (occ % 2 == 0))[0]
    pb = pa + 1
    sing = np.nonzero(~is_pair)[0]
    # pairs ordered by (segment, sb of first edge)
    p_ord = np.lexsort((sb_s[pa], seg_s[pa]))
    pa = pa[p_ord]; pb = pb[p_ord]
    ps = seg_s[pa]
    pcnt = np.bincount(ps, minlength=NSEG)
    p0 = np.zeros(NSEG, np.int64); np.cumsum(pcnt[:-1], out=p0[1:])
    pair_rank = np.arange(len(pa)) - p0[ps]
    # singles ordered by (segment, sb)
    s_ord = np.lexsort((sb_s[sing], seg_s[sing]))
    sing = sing[s_ord]
    ss = seg_s[sing]
    scnt = np.bincount(ss, minlength=NSEG)
    s0 = np.zeros(NSEG, np.int64); np.cumsum(scnt[:-1], out=s0[1:])
    sing_rank = np.arange(len(sing)) - s0[ss]

    # paired region padded to a static 128-aligned size so the singles
    # start at the same chunk-aligned offset on every core (measured better
    # than unaligned: mixed boundary chunks widen S1 spans by more than the
    # padding costs)
    pc3_ = pcnt.reshape(NCORES, NGRP, NBANK)
    sc3_ = scnt.reshape(NCORES, NGRP, NBANK)
    pseg = -(-pc3_.max(axis=0) // P) * P              # [NGRP, NBANK]
    seg_exact = pseg + sc3_.max(axis=0)
    seg = -(-seg_exact // P) * P
    seg_off = np.zeros((NGRP, NBANK), np.int64)
    pos = 0
    for g in range(NGRP):
        for b in range(NBANK):
            seg_off[g, b] = pos
            pos += int(seg[g, b])
    slots_tot = pos

    seg_off_flat = np.zeros(NSEG, np.int64)
    cseg = np.arange(NSEG)
    gg = (cseg // NBANK) % NGRP
    bb = cseg % NBANK
    seg_off_flat[:] = seg_off[gg, bb]
    c_of_seg = cseg // (NGRP * NBANK)

    # absolute per-core slot index of pairs and singles
    pseg_flat = pseg[gg, bb]
    slotA = seg_off_flat[ps] + pair_rank
    slotS = seg_off_flat[ss] + pseg_flat[ss] + sing_rank
    coreA = c_of_seg[ps]
    coreS = c_of_seg[ss]

    IDX = np.zeros((NCORES, slots_tot), np.int16)
    IDX[coreA, slotA] = srcl_s[pa]
    IDX[coreS, slotS] = srcl_s[sing]

    # chunk schedule: A-side spans from (paired-by-sbA | singles-by-sb)
    # interval marking; B-side spans per chunk from the pair second edges
    maxnch = int(seg.max()) // P
    chunkmap = np.full((NGRP, NBANK, maxnch), -1, np.int64)
    chunks = [[] for _ in range(NGRP)]
    nchunks = 0
    sc4 = np.zeros((NCORES, NGRP, NBANK, NSB), np.int64)
    np.add.at(sc4, (coreS, gg[ss], bb[ss], sb_s[sing]), 1)
    pc4 = np.zeros((NCORES, NGRP, NBANK, NSB), np.int64)
    np.add.at(pc4, (c_of_seg[ps], gg[ps], bb[ps], sb_s[pa]), 1)
    pc3 = pcnt.reshape(NCORES, NGRP, NBANK)
    # per-(g,b,chunk) B-sb min/max over all cores
    jA = pair_rank // P
    keyBJ = (gg[ps] * NBANK + bb[ps]) * maxnch + jA
    bmin = np.full(NGRP * NBANK * maxnch, NSB + 1, np.int64)
    bmax = np.full(NGRP * NBANK * maxnch, -1, np.int64)
    np.minimum.at(bmin, keyBJ, sb_s[pb])
    np.maximum.at(bmax, keyBJ, sb_s[pb])
    for g in range(NGRP):
        sbs = cfg.GROUPS[g]
        for b in range(NBANK):
            nch = int(seg[g, b]) // P
            pres = np.zeros((nch, len(sbs)), bool)
            for c in range(NCORES):
                lo = 0
                for si, s in enumerate(sbs):
                    hi = lo + int(pc4[c, g, b, s])
                    if hi > lo:
                        pres[lo // P:(hi - 1) // P + 1, si] = True
                    lo = hi
                lo = int(pseg[g, b])
                for si, s in enumerate(sbs):
                    hi = lo + int(sc4[c, g, b, s])
                    if hi > lo:
                        pres[lo // P:(hi - 1) // P + 1, si] = True
                    lo = hi
            for j in range(nch):
                (idxs,) = np.nonzero(pres[j])
                if len(idxs) == 0:
                    continue
                sb_lo = sbs[int(idxs[0])]
                nspan = int(idxs[-1]) - int(idxs[0]) + 1
                k2 = (g * NBANK + b) * maxnch + j
                if bmax[k2] >= 0:
                    sb_lo2 = int(bmin[k2])
                    nspan2 = int(bmax[k2]) - sb_lo2 + 1
                else:
                    sb_lo2, nspan2 = 0, 0
                chunkmap[g, b, j] = nchunks
                chunks[g].append((b, j, sb_lo, nspan, nchunks,
                                  sb_lo2, nspan2))
                nchunks += 1

    TLOC = np.full((NCORES, nchunks, P), SENTINEL, np.float32)
    TLOC2 = np.full((NCORES, nchunks, P), SENTINEL, np.float32)

    def _fill(cores_, slots_, vals_, arr):
        segsl = slots_  # absolute slots; chunk via per-seg offsets
        g_ = np.zeros(len(slots_), np.int64)
        b_ = np.zeros(len(slots_), np.int64)
        # recover (g,b) from slot via searchsorted on seg_off boundaries
        bounds = np.array([int(seg_off[gx, bx]) for gx in range(NGRP)
                           for bx in range(NBANK)] + [slots_tot])
        si_ = np.searchsorted(bounds, slots_, side="right") - 1
        g_ = si_ // NBANK
        b_ = si_ % NBANK
        j_ = (slots_ - bounds[si_]) // P
        pos_ = (slots_ - bounds[si_]) % P
        ci_ = chunkmap[g_, b_, j_]
        assert (ci_ >= 0).all()
        arr[cores_, ci_, pos_] = vals_

    _fill(coreA, slotA, tlg_s[pa], TLOC)
    _fill(coreA, slotA, tlg_s[pb], TLOC2)
    _fill(coreS, slotS, tlg_s[sing], TLOC)
    EWA = None
    if not ew_ones:
        raise NotImplementedError("pairing path assumes unit edge weights")

    # pack gather indices: wrap-16, replicate to 128 partitions
    assert slots_tot % 16 == 0
    idx_packed = IDX.reshape(NCORES, slots_tot // 16, 16).transpose(0, 2, 1)
    idx_all = np.tile(idx_packed, (1, 8, 1)).astype(np.int16)

    tloc_all = TLOC.transpose(0, 2, 1).copy()         # [NCORES, P, ncols]
    tloc2_all = TLOC2.transpose(0, 2, 1).copy()
    ew_all = None if EWA is None else EWA.transpose(0, 2, 1).copy()

    iota16 = np.tile(np.arange(SBG * P, dtype=np.float16), (P, 1))
    ident16 = np.eye(P, dtype=np.float16)

    sched = dict(chunks=chunks, seg=seg, seg_exact=seg_exact,
                 seg_off=seg_off, slots_tot=slots_tot,
                 nchunks=nchunks, ew_ones=ew_ones)
    in_maps = []
    for c in range(NCORES):
        m = dict(
            embs16=embs16,
            w32=np.asarray(W, np.float32),
            gxT=np.ascontiguousarray(gX[c * TPC:(c + 1) * TPC].T),
            selfT=np.ascontiguousarray(embs16[c * TPC:(c + 1) * TPC].T),
            idxall=np.ascontiguousarray(idx_all[c]),
            tlocall=np.ascontiguousarray(tloc_all[c]),
            tloc2all=np.ascontiguousarray(tloc2_all[c]),
            iota16=iota16,
            ident16=ident16,
        )
        if ew_all is not None:
            m["ewall"] = np.ascontiguousarray(ew_all[c])
        in_maps.append(m)
    return sched, in_maps


def _build_program(cfg, sched):
    N, TPC, NSB, BANK, NBANK, SBG, NGRP = (
        cfg.N, cfg.TPC, cfg.NSB, cfg.BANK, cfg.NBANK, cfg.SBG, cfg.NGRP)
    chunks, seg, seg_exact, seg_off, slots_tot, nchunks, ew_ones = (
        sched["chunks"], sched["seg"], sched["seg_exact"], sched["seg_off"],
        sched["slots_tot"], sched["nchunks"], sched["ew_ones"])

    nc = bacc.Bacc("TRN2", target_bir_lowering=False, debug=False,
                   num_devices=cfg.NCORES)
    t_embs16 = nc.dram_tensor("embs16", [N, P], mybir.dt.float16,
                              kind="ExternalInput").ap()
    t_w = nc.dram_tensor("w32", [P, P], mybir.dt.float32,
                         kind="ExternalInput").ap()
    t_gxT = nc.dram_tensor("gxT", [P, TPC], mybir.dt.float16,
                           kind="ExternalInput").ap()
    t_selfT = nc.dram_tensor("selfT", [P, TPC], mybir.dt.float16,
                             kind="ExternalInput").ap()
    t_idx = nc.dram_tensor("idxall", [P, slots_tot // 16], mybir.dt.int16,
                           kind="ExternalInput").ap()
    t_tloc = nc.dram_tensor("tlocall", [P, nchunks], mybir.dt.float32,
                            kind="ExternalInput").ap()
    t_tloc2 = nc.dram_tensor("tloc2all", [P, nchunks], mybir.dt.float32,
                             kind="ExternalInput").ap()
    t_iota = nc.dram_tensor("iota16", [P, SBG * P], mybir.dt.float16,
                            kind="ExternalInput").ap()
    t_ident = nc.dram_tensor("ident16", [P, P], mybir.dt.float16,
                             kind="ExternalInput").ap()
    t_ew = None
    if not ew_ones:
        t_ew = nc.dram_tensor("ewall", [P, nchunks], mybir.dt.float32,
                              kind="ExternalInput").ap()
    t_outT = nc.dram_tensor("outT", [P, TPC], mybir.dt.float16,
                            kind="ExternalOutput").ap()

    with tile.TileContext(nc) as tc:
        with tc.tile_pool(name="const", bufs=1) as cpool, \
             tc.tile_pool(name="meta", bufs=1) as mpool, \
             tc.tile_pool(name="gpool", bufs=8) as gpool, \
             tc.tile_pool(name="spool", bufs=12) as spool, \
             tc.tile_pool(name="grp", bufs=3) as grp, \
             tc.tile_pool(name="xfer", bufs=6) as xfer, \
             tc.tile_pool(name="psu", bufs=4, space="PSUM") as psu, \
             tc.tile_pool(name="psb", bufs=4, space="PSUM") as psb:

            iota_t = cpool.tile([P, SBG * P], mybir.dt.float16)
            nc.sync.dma_start(out=iota_t, in_=t_iota)
            zeros_t = cpool.tile([P, 4 * P], mybir.dt.float16)
            nc.vector.memset(zeros_t[:, :], 0.0)
            ident_t = cpool.tile([P, P], mybir.dt.float16)
            nc.gpsimd.dma_start(out=ident_t, in_=t_ident)
            w_t = cpool.tile([P, P], mybir.dt.float32)
            nc.gpsimd.dma_start(out=w_t, in_=t_w)
            # idx loaded in pieces (separate tiles): a tiny head covering
            # only the first (group, bank) segment lets the first gather's
            # SWDGE generation start ~3us earlier; the rest follows in
            # group-aligned slabs behind tloc
            idx_cuts = [0, int(seg[0, 0]) // 16,
                        int(seg_off[2, 0]) // 16,
                        int(seg_off[6, 0]) // 16,
                        slots_tot // 16]
            idx_ts = []
            q0 = mpool.tile([P, idx_cuts[1]], mybir.dt.int16, name="idx_q0")
            nc.gpsimd.dma_start(out=q0, in_=t_idx[:, 0:idx_cuts[1]])
            idx_ts.append((0, q0))
            tloc_t = mpool.tile([P, nchunks], mybir.dt.float32)
            nc.gpsimd.dma_start(out=tloc_t, in_=t_tloc)
            tloc2_t = mpool.tile([P, nchunks], mybir.dt.float32)
            nc.gpsimd.dma_start(out=tloc2_t, in_=t_tloc2)
            for qi in range(1, len(idx_cuts) - 1):
                lo, hi = idx_cuts[qi], idx_cuts[qi + 1]
                q_t = mpool.tile([P, hi - lo], mybir.dt.int16,
                                 name=f"idx_q{qi}")
                nc.sync.dma_start(out=q_t, in_=t_idx[:, lo:hi])
                idx_ts.append((lo, q_t))

            def _idx_slice(o16, n16):
                for qi in range(len(idx_ts) - 1, -1, -1):
                    lo, q_t = idx_ts[qi]
                    if o16 >= lo:
                        return q_t[:, o16 - lo:o16 - lo + n16]
                raise AssertionError
            ew_t = None
            if t_ew is not None:
                ew_t = mpool.tile([P, nchunks], mybir.dt.float32)
                nc.sync.dma_start(out=ew_t, in_=t_ew)

            nch_max = int(seg.max()) // P
            for wi in range(8):
                warm_t = gpool.tile([P, nch_max, P], mybir.dt.float16,
                                    tag="g", name=f"warm_{wi}")
                nc.vector.memset(warm_t[:, :, :], 0.0)


            for g in range(NGRP):
                sbs = cfg.GROUPS[g]
                g0 = cfg.GSTART[g]
                t0g = g0 * P
                wg = min(len(sbs) * P, TPC - t0g)

                g_tiles = []
                for b in range(NBANK):
                    nch = int(seg[g, b]) // P
                    g_t = gpool.tile([P, nch, P], mybir.dt.float16, tag="g")
                    o16 = int(seg_off[g, b]) // 16
                    exact = int(seg_exact[g, b])
                    assert 0 < exact <= nch * P
                    # the very last segment is fetched in three pieces so the
                    # tail chunks' matmuls overlap the remaining transfers
                    npc = 3 if (g == NGRP - 1 and b == NBANK - 1) else 1
                    ccuts = sorted({0} | {min((-(-nch // npc)) * (i + 1), nch)
                                          for i in range(npc)})
                    for c0, c1 in zip(ccuts[:-1], ccuts[1:]):
                        ni = min(exact, c1 * P) - c0 * P
                        if ni <= 0:
                            continue
                        rows = min(BANK, N - b * BANK)
                        nc.gpsimd.dma_gather(
                            out_ap=g_t[:, c0:c1, :],
                            in_ap=t_embs16[b * BANK: b * BANK + rows, :],
                            idxs_ap=_idx_slice(o16 + c0 * 8, -(-ni // 16)),
                            num_idxs=ni,
                            num_idxs_reg=ni,
                            elem_size=P,
                            single_packet=False,
                        )
                    g_tiles.append(g_t)

                # stream loads AFTER the gather issues: they are only
                # consumed by this group's tail, and keeping them behind the
                # gathers in the DMA queue lets the final gathers finish
                # earlier
                selfT_t = grp.tile([P, wg], mybir.dt.float16, tag="self")
                nc.sync.dma_start(out=selfT_t, in_=t_selfT[:, t0g:t0g + wg])
                gxT_t = grp.tile([P, wg], mybir.dt.float16, tag="gx")
                nc.sync.dma_start(out=gxT_t, in_=t_gxT[:, t0g:t0g + wg])
                outT_t = grp.tile([P, wg], mybir.dt.float16, tag="out")

                nhalf = -(-len(sbs) // 4)
                pu_t = []
                for h in range(nhalf):
                    p_t = psu.tile([P, 4 * P], mybir.dt.float32, space="PSUM",
                                   tag="pu", name=f"psu_{g}_{h}")
                    # open the bank's single accumulation group, zeroing all
                    # four 128-col windows (PSUM groups are bank-granular)
                    nc.tensor.matmul(out=p_t[:, :], lhsT=ident_t,
                                     rhs=zeros_t, start=True, stop=False)
                    pu_t.append(p_t)

                def _uwin(s, pu_t=pu_t, g0=g0):
                    sbl = s - g0
                    return pu_t[sbl // 4], (sbl % 4) * P

                # last sb of each bank closes the group (stop=True)
                last_sb_of_half = {h: sbs[min(4 * h + 4, len(sbs)) - 1]
                                   for h in range(nhalf)}



                for (b, j, sb_lo, nspan, ci, sb_lo2, nspan2) in chunks[g]:
                    w0 = (sb_lo - g0) * P
                    ws = nspan * P
                    s_t = spool.tile([P, ws], mybir.dt.float16, tag="s",
                                     name=f"s_{g}_{b}_{j}")
                    nc.vector.tensor_scalar(
                        out=s_t, in0=iota_t[:, w0:w0 + ws],
                        scalar1=tloc_t[:, ci:ci + 1], scalar2=None,
                        op0=mybir.AluOpType.is_equal)
                    for k in range(nspan):
                        s = sb_lo + k
                        tw = min(P, TPC - s * P)
                        put, uoff = _uwin(s)
                        nc.tensor.matmul(
                            out=put[:, uoff:uoff + tw],
                            lhsT=g_tiles[b][:, j, :],
                            rhs=s_t[:, k * P:k * P + tw],
                            start=False, stop=False)
                    if nspan2 > 0:
                        w02 = (sb_lo2 - g0) * P
                        ws2 = nspan2 * P
                        s2_t = spool.tile([P, ws2], mybir.dt.float16,
                                          tag="s", name=f"s2_{g}_{b}_{j}")
                        nc.vector.tensor_scalar(
                            out=s2_t, in0=iota_t[:, w02:w02 + ws2],
                            scalar1=tloc2_t[:, ci:ci + 1], scalar2=None,
                            op0=mybir.AluOpType.is_equal)
                        for k in range(nspan2):
                            s = sb_lo2 + k
                            tw = min(P, TPC - s * P)
                            put, uoff = _uwin(s)
                            nc.tensor.matmul(
                                out=put[:, uoff:uoff + tw],
                                lhsT=g_tiles[b][:, j, :],
                                rhs=s2_t[:, k * P:k * P + tw],
                                start=False, stop=False)

                def _tail(g, sbs, _uwin, last_sb_of_half, selfT_t,
                          gxT_t, outT_t, t0g, wg, g0):
                    for s in sbs:
                        tw = min(P, TPC - s * P)
                        sbl = s - g0
                        # self loops: += embs'^T[:, t] via identity matmul
                        put, uoff = _uwin(s)
                        h = sbl // 4
                        nc.tensor.matmul(
                            out=put[:, uoff:uoff + tw],
                            lhsT=ident_t,
                            rhs=selfT_t[:, sbl * P: sbl * P + tw],
                            start=False, stop=(s == last_sb_of_half[h]))

                        u_t = xfer.tile([P, P], mybir.dt.float32, tag="u")
                        if g == NGRP - 1 and sbl % 2:
                            nc.vector.tensor_copy(out=u_t[:, :tw],
                                                  in_=put[:, uoff:uoff + tw])
                        else:
                            nc.scalar.copy(out=u_t[:, :tw],
                                           in_=put[:, uoff:uoff + tw])

                        pot = psb.tile([P, P], mybir.dt.float32, space="PSUM",
                                       tag="po", name=f"pso_{g}_{s}")
                        nc.tensor.matmul(out=pot[:, :tw], lhsT=w_t,
                                         rhs=u_t[:, :tw], start=True,
                                         stop=True)

                        nc.vector.tensor_tensor(
                            out=outT_t[:, sbl * P: sbl * P + tw],
                            in0=pot[:, :tw],
                            in1=gxT_t[:, sbl * P: sbl * P + tw],
                            op=mybir.AluOpType.mult)

                    nc.sync.dma_start(out=t_outT[:, t0g:t0g + wg],
                                      in_=outT_t)

                _tail(g, sbs, _uwin, last_sb_of_half, selfT_t, gxT_t,
                      outT_t, t0g, wg, g0)
    nc.compile()
    return nc


def kernel(X, embs, W, edge_index, edge_weight):
    cfg = _REAL
    sched, in_maps = _host_prep(cfg, X, embs, W, edge_index, edge_weight)
    nc = _build_program(cfg, sched)
    res = run_bass_kernel_spmd(nc, in_maps, list(range(cfg.NCORES)))
    out = np.concatenate(
        [res.results[c]["outT"].T for c in range(cfg.NCORES)], axis=0)
    return out.astype(np.float32)
